# revision 9
# baseline (speedup 1.0000x reference)
"""Trainium2 Bass kernel for nn_Net_25984552141404 (ragged_sequence).

Forward pass: token embedding gather -> masked mean + first-token feature ->
3-layer MLP. Data-parallel over 8 NeuronCores (batch 8192 -> 1024 rows/core).

Key device-side structure (per core):
- x arrives pre-transposed as xt [1024, 128] int32 (host-side layout prep).
- For each 128-row b-tile, the sequence-sum of embeddings is computed by
  indirect DMA gathers (one per sequence position) with compute_op=add:
  the SDMA CCE unit accumulates bf16 table rows into f32 SBUF accumulators.
- Ragged masking: for s > last-nonpad, tokens are PAD, so
  masked_sum = full_sum - pad_count * emb[PAD]. pad_count comes from a
  suffix-OR mask computed with log-tree max ops on the vector engine.
- feat = [emb(token at s=0), masked_mean] is transposed via TensorE into
  [600, batch] bf16 and fed through the MLP with weights SBUF-resident.
- Layer 3 uses activations as the stationary operand so the output lands
  as [batch, class] for contiguous output DMA.
"""
import os
import sys

import numpy as np

try:
    import concourse  # noqa: F401
except ImportError:
    sys.path.insert(0, "/opt/trn_rl_repo")

import ml_dtypes

import concourse.bass as bass
import concourse.tile as tile
from concourse import bacc, mybir
from concourse.bass_utils import run_bass_kernel_spmd
from concourse.masks import make_identity

S = 128
B = 8192
E = 300
V = 100000
H1 = 2048
H2 = 2048
C = 1221
PAD = 1
NCORES = 8
BC = B // NCORES           # 1024 rows per core
NBT = BC // 128            # 8 b-tiles per core
NCH = 256                  # MLP batch chunk (free dim)
NCHUNKS = BC // NCH
ACC_ROT = 4                # gather accumulator rotation depth
# feat packed into 5 k-tiles of 128 partitions; tile 2 holds ex0[256:300] at
# partitions 0-43 and mean[0:64] at partitions 64-127 (gap rows are zero in
# the packed W1T, so they contribute nothing).
KT1 = [(0, 128), (128, 128), (256, 128), (384, 128), (512, 108)]
W1P = 640                  # packed W1T row count (5 * 128)
CCH = [(0, 512), (512, 512), (1024, C - 1024)]                   # class chunks

bf16 = ml_dtypes.bfloat16
_nc_cache = {}


def build():
    if "nc" in _nc_cache:
        return _nc_cache["nc"]
    f32 = mybir.dt.float32
    bf = mybir.dt.bfloat16
    i32 = mybir.dt.int32

    nc = bacc.Bacc("TRN2", target_bir_lowering=False, debug=False,
                   num_devices=NCORES)
    xt_d = nc.dram_tensor("xt", [BC, S], i32, kind="ExternalInput")
    emb_d = nc.dram_tensor("emb", [V, E], bf, kind="ExternalInput")
    w1t_d = nc.dram_tensor("w1t", [W1P, H1], bf, kind="ExternalInput")
    w2t_d = nc.dram_tensor("w2t", [H1, H2], bf, kind="ExternalInput")
    w3t_d = nc.dram_tensor("w3t", [H2, C], bf, kind="ExternalInput")
    b1_d = nc.dram_tensor("b1s", [128, H1 // 128], f32, kind="ExternalInput")
    b2_d = nc.dram_tensor("b2s", [128, H2 // 128], f32, kind="ExternalInput")
    bo_d = nc.dram_tensor("bor", [128, C], f32, kind="ExternalInput")
    pr_d = nc.dram_tensor("padr", [128, E], f32, kind="ExternalInput")
    out_d = nc.dram_tensor("out", [BC, C], f32, kind="ExternalOutput")

    with tile.TileContext(nc) as tc:
        with tc.tile_pool(name="w", bufs=1) as wp, \
             tc.tile_pool(name="acc", bufs=2 * ACC_ROT) as ap_, \
             tc.tile_pool(name="bt", bufs=2) as bp, \
             tc.tile_pool(name="h", bufs=1) as hp, \
             tc.tile_pool(name="o", bufs=2) as op_, \
             tc.tile_pool(name="mm", bufs=4, space="PSUM") as mp, \
             tc.tile_pool(name="tp", bufs=2, space="PSUM") as tpp, \
             tc.tile_pool(name="p3", bufs=2, space="PSUM") as p3p:

            # ---- persistent loads -------------------------------------
            w1s = wp.tile([128, len(KT1), H1], bf)
            for kt in range(len(KT1)):
                nc.sync.dma_start(out=w1s[:, kt, :],
                                  in_=w1t_d[kt * 128:(kt + 1) * 128, :])
            w2s = wp.tile([128, H1 // 128, H2], bf)
            for kt in range(H1 // 128):
                nc.sync.dma_start(out=w2s[:, kt, :],
                                  in_=w2t_d[kt * 128:(kt + 1) * 128, :])
            w3s = wp.tile([128, H2 // 128, C], bf)
            for kt in range(H2 // 128):
                nc.sync.dma_start(out=w3s[:, kt, :],
                                  in_=w3t_d[kt * 128:(kt + 1) * 128, :])
            b1s = wp.tile([128, H1 // 128], f32)
            nc.sync.dma_start(out=b1s[:], in_=b1_d[:])
            b2s = wp.tile([128, H2 // 128], f32)
            nc.sync.dma_start(out=b2s[:], in_=b2_d[:])
            bos = wp.tile([128, C], f32)
            nc.sync.dma_start(out=bos[:], in_=bo_d[:])
            prs = wp.tile([128, E], f32)
            nc.sync.dma_start(out=prs[:], in_=pr_d[:])
            idf = wp.tile([128, 128], f32)
            make_identity(nc, idf[:])

            xt = wp.tile([128, NBT, S], i32)
            nc.sync.dma_start(
                out=xt[:], in_=xt_d[:].rearrange("(t p) s -> p t s", p=128))

            # ---- ragged mask: suffix-OR of nonpad, then counts --------
            msk = wp.tile([128, NBT, S], bf)
            nc.vector.tensor_copy(out=msk[:], in_=xt[:])  # int32 -> bf16
            nc.vector.tensor_scalar(out=msk[:], in0=msk[:], scalar1=float(PAD),
                                    scalar2=None, op0=mybir.AluOpType.not_equal)
            k = 1
            while k < S:
                nc.vector.tensor_tensor(out=msk[:, :, :S - k],
                                        in0=msk[:, :, :S - k],
                                        in1=msk[:, :, k:],
                                        op=mybir.AluOpType.max)
                k *= 2
            cnt = wp.tile([128, NBT], f32)
            for t in range(NBT):
                nc.vector.tensor_reduce(out=cnt[:, t:t + 1], in_=msk[:, t, :],
                                        axis=mybir.AxisListType.X,
                                        op=mybir.AluOpType.add)
            ncd = wp.tile([128, NBT], f32)   # cnt - S  (= -pad_count)
            nc.vector.tensor_scalar(out=ncd[:], in0=cnt[:], scalar1=-float(S),
                                    scalar2=None, op0=mybir.AluOpType.add)
            inv = wp.tile([128, NBT], f32)   # 1 / cnt
            nc.vector.reciprocal(out=inv[:], in_=cnt[:])

            featT = wp.tile([128, len(KT1), BC], bf)
            nc.vector.memset(featT[:], 0.0)

            # transposes of [128, w]->[w, 128] chunks into featT k-tiles
            # (src_sel, col0, width, ktile, row0): src 0 = ex0, 1 = mean
            tplan = [(0, 0, 128, 0, 0), (0, 128, 128, 1, 0), (0, 256, 44, 2, 0),
                     (1, 0, 64, 2, 64), (1, 64, 128, 3, 0), (1, 192, 108, 4, 0)]

            for t in range(NBT):
                # ---- gather-accumulate embeddings over s ----------------
                accs = []
                for r in range(ACC_ROT):
                    a = ap_.tile([128, E], f32, tag="acc")
                    for s in range(r, S, ACC_ROT):
                        nc.gpsimd.indirect_dma_start(
                            out=a[:], out_offset=None, in_=emb_d[:],
                            in_offset=bass.IndirectOffsetOnAxis(
                                ap=xt[:, t, s:s + 1], axis=0),
                            compute_op=(mybir.AluOpType.bypass if s == r
                                        else mybir.AluOpType.add))
                    accs.append(a)
                ex0 = bp.tile([128, E], f32, tag="ex0")
                nc.gpsimd.indirect_dma_start(
                    out=ex0[:], out_offset=None, in_=emb_d[:],
                    in_offset=bass.IndirectOffsetOnAxis(
                        ap=xt[:, t, 0:1], axis=0))
                # combine rotated accumulators -> sum over all s
                nc.vector.tensor_tensor(out=accs[0][:], in0=accs[0][:],
                                        in1=accs[1][:], op=mybir.AluOpType.add)
                nc.vector.tensor_tensor(out=accs[2][:], in0=accs[2][:],
                                        in1=accs[3][:], op=mybir.AluOpType.add)
                sm = bp.tile([128, E], f32, tag="sum")
                nc.vector.tensor_tensor(out=sm[:], in0=accs[0][:],
                                        in1=accs[2][:], op=mybir.AluOpType.add)
                # masked sum = full sum + (cnt - S) * emb[PAD]; mean = /cnt
                nc.vector.scalar_tensor_tensor(
                    out=sm[:], in0=prs[:], scalar=ncd[:, t:t + 1], in1=sm[:],
                    op0=mybir.AluOpType.mult, op1=mybir.AluOpType.add)
                nc.vector.tensor_scalar(out=sm[:], in0=sm[:],
                                        scalar1=inv[:, t:t + 1], scalar2=None,
                                        op0=mybir.AluOpType.mult)
                # ---- feat transposes into featT -------------------------
                for (srcsel, c0, w, kt, r0) in tplan:
                    src = ex0 if srcsel == 0 else sm
                    tp = tpp.tile([128, 128], f32, tag="tp")
                    nc.tensor.transpose(out=tp[:w, :], in_=src[:, c0:c0 + w],
                                        identity=idf[:])
                    nc.vector.tensor_copy(
                        out=featT[r0:r0 + w, kt, t * 128:(t + 1) * 128],
                        in_=tp[:w, :])

            # ---- MLP ---------------------------------------------------
            for n in range(NCHUNKS):
                ns = slice(n * NCH, (n + 1) * NCH)
                h1 = hp.tile([128, H1 // 128, NCH], bf, tag="h1")
                for m in range(H1 // 128):
                    ps = mp.tile([128, NCH], f32, tag="mm")
                    for kt, (k0, ksz) in enumerate(KT1):
                        kn = 128 if kt < 4 else ksz
                        nc.tensor.matmul(out=ps[:],
                                         lhsT=w1s[:kn, kt, m * 128:(m + 1) * 128],
                                         rhs=featT[:kn, kt, ns],
                                         start=(kt == 0), stop=(kt == len(KT1) - 1))
                    nc.vector.tensor_scalar(out=h1[:, m, :], in0=ps[:],
                                            scalar1=b1s[:, m:m + 1], scalar2=0.0,
                                            op0=mybir.AluOpType.add,
                                            op1=mybir.AluOpType.max)
                h2 = hp.tile([128, H2 // 128, NCH], bf, tag="h2")
                for m in range(H2 // 128):
                    ps = mp.tile([128, NCH], f32, tag="mm")
                    for kt in range(H1 // 128):
                        nc.tensor.matmul(out=ps[:],
                                         lhsT=w2s[:, kt, m * 128:(m + 1) * 128],
                                         rhs=h1[:, kt, :],
                                         start=(kt == 0), stop=(kt == H1 // 128 - 1))
                    nc.vector.tensor_scalar(out=h2[:, m, :], in0=ps[:],
                                            scalar1=b2s[:, m:m + 1], scalar2=0.0,
                                            op0=mybir.AluOpType.add,
                                            op1=mybir.AluOpType.max)
                # layer 3: activations stationary -> out [b, class]
                for bt in range(NCH // 128):
                    t = n * (NCH // 128) + bt
                    osb = op_.tile([128, C], f32, tag="osb")
                    for (c0, cw) in CCH:
                        ps3 = p3p.tile([128, 512], f32, tag="p3")
                        for kt in range(H2 // 128):
                            nc.tensor.matmul(
                                out=ps3[:, :cw],
                                lhsT=h2[:, kt, bt * 128:(bt + 1) * 128],
                                rhs=w3s[:, kt, c0:c0 + cw],
                                start=(kt == 0), stop=(kt == H2 // 128 - 1))
                        nc.vector.tensor_tensor(out=osb[:, c0:c0 + cw],
                                                in0=ps3[:, :cw],
                                                in1=bos[:, c0:c0 + cw],
                                                op=mybir.AluOpType.add)
                    nc.sync.dma_start(out=out_d[t * 128:(t + 1) * 128, :],
                                      in_=osb[:])
    nc.compile()
    _nc_cache["nc"] = nc
    return nc


def kernel(x, emb, W1, b1, W2, b2, W_out, b_out):
    x = np.asarray(x)
    emb = np.asarray(emb, dtype=np.float32)
    embb = emb.astype(bf16)
    # pack W1T rows to match the featT k-tile layout (zeros in the seam gap)
    w1t_nat = np.asarray(W1, np.float32).T  # [600, H1]
    w1t_pk = np.zeros((W1P, H1), np.float32)
    w1t_pk[0:300] = w1t_nat[0:300]          # ex0 rows (k-tiles 0-2)
    w1t_pk[320:384] = w1t_nat[300:364]      # mean[0:64] at tile2 partition 64
    w1t_pk[384:512] = w1t_nat[364:492]      # mean[64:192] -> tile 3
    w1t_pk[512:620] = w1t_nat[492:600]      # mean[192:300] -> tile 4
    w1t = np.ascontiguousarray(w1t_pk.astype(bf16))
    w2t = np.ascontiguousarray(np.asarray(W2, np.float32).T.astype(bf16))
    w3t = np.ascontiguousarray(np.asarray(W_out, np.float32).T.astype(bf16))
    b1s = np.ascontiguousarray(
        np.asarray(b1, np.float32).reshape(H1 // 128, 128).T)
    b2s = np.ascontiguousarray(
        np.asarray(b2, np.float32).reshape(H2 // 128, 128).T)
    bor = np.ascontiguousarray(
        np.broadcast_to(np.asarray(b_out, np.float32), (128, C)))
    padr = np.ascontiguousarray(
        np.broadcast_to(embb[PAD].astype(np.float32), (128, E)))
    xt = np.ascontiguousarray(x.T.astype(np.int32))  # [B, S]

    nc = build()
    in_maps = [{
        "xt": xt[c * BC:(c + 1) * BC],
        "emb": embb, "w1t": w1t, "w2t": w2t, "w3t": w3t,
        "b1s": b1s, "b2s": b2s, "bor": bor, "padr": padr,
    } for c in range(NCORES)]
    res = run_bass_kernel_spmd(nc, in_maps, core_ids=list(range(NCORES)))
    return np.concatenate([res.results[c]["out"] for c in range(NCORES)],
                          axis=0)


# revision 23
# speedup vs baseline: 1.0212x; 1.0212x over previous
"""Trainium2 Bass kernel for nn_Net_25984552141404 (ragged_sequence).

Forward pass: token embedding gather -> masked mean + first-token feature ->
3-layer MLP. Data-parallel over 8 NeuronCores (batch 8192 -> 1024 rows/core).

Key device-side structure (per core):
- x arrives pre-transposed as xt [1024, 128] int32 (host-side layout prep).
- For each 128-row b-tile, the sequence-sum of embeddings is computed by
  indirect DMA gathers (one per sequence position) with compute_op=add:
  the SDMA CCE unit accumulates bf16 table rows into f32 SBUF accumulators.
- Ragged masking: for s > last-nonpad, tokens are PAD, so
  masked_sum = full_sum - pad_count * emb[PAD]. pad_count comes from a
  suffix-OR mask computed with log-tree max ops on the vector engine.
- feat = [emb(token at s=0), masked_mean] is transposed via TensorE into
  [600, batch] bf16 and fed through the MLP with weights SBUF-resident.
- Layer 3 uses activations as the stationary operand so the output lands
  as [batch, class] for contiguous output DMA.
"""
import os
import sys

import numpy as np

try:
    import concourse  # noqa: F401
except ImportError:
    sys.path.insert(0, "/opt/trn_rl_repo")

import ml_dtypes

import concourse.bass as bass
import concourse.tile as tile
from concourse import bacc, mybir
from concourse.bass_utils import run_bass_kernel_spmd
from concourse.masks import make_identity

S = 128
B = 8192
E = 300
V = 100000
H1 = 2048
H2 = 2048
C = 1221
PAD = 1
NCORES = 8
BC = B // NCORES           # 1024 rows per core
NBT = BC // 128            # 8 b-tiles per core
NCH = 256                  # MLP batch chunk (free dim)
NCHUNKS = BC // NCH
ACC_ROT = int(os.environ.get("K_ACC_ROT", "8"))  # gather accumulator rotation
SKIP_MLP = os.environ.get("K_SKIP_MLP") == "1"
SKIP_GATHER = os.environ.get("K_SKIP_GATHER") == "1"
USE_DMA_GATHER = os.environ.get("K_DMA_GATHER", "1") == "1"

# --- dma_gather table layout: zero rows interleaved every W vocab rows ---
# table position of vocab row v is v + 1 + v//W; positions r*(W+1) are zero
# rows, so clamping a window-local index to [0, hi] maps out-of-window
# tokens onto zero rows.
W = 32766                  # real rows per window (int16 positive range - 1)
EP = 384                   # padded row elems (768B, 256B-aligned)
NR = V + V // W + 2        # 100005 padded-table rows
NWIN = (V + W - 1) // W    # 4 windows
GNS = 16                   # s-steps per dma_gather call (num_idxs = 2048)
# feat packed into 5 k-tiles of 128 partitions; tile 2 holds ex0[256:300] at
# partitions 0-43 and mean[0:64] at partitions 64-127 (gap rows are zero in
# the packed W1T, so they contribute nothing).
KT1 = [(0, 128), (128, 128), (256, 128), (384, 128), (512, 108)]
W1P = 640                  # packed W1T row count (5 * 128)
CCH = [(0, 512), (512, 512), (1024, C - 1024)]                   # class chunks

bf16 = ml_dtypes.bfloat16
_nc_cache = {}


def build():
    if "nc" in _nc_cache:
        return _nc_cache["nc"]
    f32 = mybir.dt.float32
    bf = mybir.dt.bfloat16
    i32 = mybir.dt.int32

    nc = bacc.Bacc("TRN2", target_bir_lowering=False, debug=False,
                   num_devices=NCORES)
    i16 = mybir.dt.int16
    xt_d = nc.dram_tensor("xt", [BC, S], i32, kind="ExternalInput")
    emb_d = nc.dram_tensor("emb", [V, E], bf, kind="ExternalInput")
    if USE_DMA_GATHER:
        embp_d = nc.dram_tensor("embp", [NR, EP], bf, kind="ExternalInput")
        xg_d = nc.dram_tensor("xg", [NBT, 128, 8 * S], i32, kind="ExternalInput")
    w1t_d = nc.dram_tensor("w1t", [W1P, H1], bf, kind="ExternalInput")
    w2t_d = nc.dram_tensor("w2t", [H1, H2], bf, kind="ExternalInput")
    w3t_d = nc.dram_tensor("w3t", [H2, C], bf, kind="ExternalInput")
    b1_d = nc.dram_tensor("b1s", [128, H1 // 128], f32, kind="ExternalInput")
    b2_d = nc.dram_tensor("b2s", [128, H2 // 128], f32, kind="ExternalInput")
    bo_d = nc.dram_tensor("bor", [128, C], f32, kind="ExternalInput")
    pr_d = nc.dram_tensor("padr", [128, E], f32, kind="ExternalInput")
    out_d = nc.dram_tensor("out", [BC, C], f32, kind="ExternalOutput")

    with tile.TileContext(nc) as tc:
        with tc.tile_pool(name="w", bufs=1) as wp, \
             tc.tile_pool(name="acc", bufs=(2 if USE_DMA_GATHER
                                            else 2 * ACC_ROT)) as ap_, \
             tc.tile_pool(name="bt", bufs=2) as bp, \
             tc.tile_pool(name="c1", bufs=1) as cp, \
             tc.tile_pool(name="h", bufs=1) as hp, \
             tc.tile_pool(name="o", bufs=1) as op_, \
             tc.tile_pool(name="mm", bufs=3, space="PSUM") as mp, \
             tc.tile_pool(name="sm", bufs=1, space="PSUM") as smpool, \
             tc.tile_pool(name="tp", bufs=2, space="PSUM") as tpp, \
             tc.tile_pool(name="p3", bufs=2, space="PSUM") as p3p:

            # ---- persistent loads -------------------------------------
            w1s = wp.tile([128, len(KT1), H1], bf)
            for kt in range(len(KT1)):
                nc.sync.dma_start(out=w1s[:, kt, :],
                                  in_=w1t_d[kt * 128:(kt + 1) * 128, :])
            w2s = wp.tile([128, H1 // 128, H2], bf)
            for kt in range(H1 // 128):
                nc.sync.dma_start(out=w2s[:, kt, :],
                                  in_=w2t_d[kt * 128:(kt + 1) * 128, :])
            w3s = wp.tile([128, H2 // 128, C], bf)
            for kt in range(H2 // 128):
                nc.sync.dma_start(out=w3s[:, kt, :],
                                  in_=w3t_d[kt * 128:(kt + 1) * 128, :])
            b1s = wp.tile([128, H1 // 128], f32)
            nc.sync.dma_start(out=b1s[:], in_=b1_d[:])
            b2s = wp.tile([128, H2 // 128], f32)
            nc.sync.dma_start(out=b2s[:], in_=b2_d[:])
            bos = wp.tile([128, C], f32)
            nc.sync.dma_start(out=bos[:], in_=bo_d[:])
            prs = wp.tile([128, E], f32)
            nc.sync.dma_start(out=prs[:], in_=pr_d[:])
            idf = wp.tile([128, 128], f32)
            make_identity(nc, idf[:])

            xt = wp.tile([128, NBT, S], i32)
            nc.sync.dma_start(
                out=xt[:], in_=xt_d[:].rearrange("(t p) s -> p t s", p=128))

            # ---- ragged mask: suffix-OR of nonpad, then counts --------
            msk = wp.tile([128, NBT, S], bf)
            nc.vector.tensor_copy(out=msk[:], in_=xt[:])  # int32 -> bf16
            nc.vector.tensor_scalar(out=msk[:], in0=msk[:], scalar1=float(PAD),
                                    scalar2=None, op0=mybir.AluOpType.not_equal)
            k = 1
            while k < S:
                nc.vector.tensor_tensor(out=msk[:, :, :S - k],
                                        in0=msk[:, :, :S - k],
                                        in1=msk[:, :, k:],
                                        op=mybir.AluOpType.max)
                k *= 2
            cnt = wp.tile([128, NBT], f32)
            for t in range(NBT):
                nc.vector.tensor_reduce(out=cnt[:, t:t + 1], in_=msk[:, t, :],
                                        axis=mybir.AxisListType.X,
                                        op=mybir.AluOpType.add)
            ncd = wp.tile([128, NBT], f32)   # cnt - S  (= -pad_count)
            nc.vector.tensor_scalar(out=ncd[:], in0=cnt[:], scalar1=-float(S),
                                    scalar2=None, op0=mybir.AluOpType.add)
            inv = wp.tile([128, NBT], f32)   # 1 / cnt
            nc.vector.reciprocal(out=inv[:], in_=cnt[:])

            featT = wp.tile([128, len(KT1), BC], bf)
            nc.vector.memset(featT[:], 0.0)

            # transposes of [128, w]->[w, 128] chunks into featT k-tiles
            # (src_sel, col0, width, ktile, row0): src 0 = ex0, 1 = mean
            tplan = [(0, 0, 128, 0, 0), (0, 128, 128, 1, 0), (0, 256, 44, 2, 0),
                     (1, 0, 64, 2, 64), (1, 64, 128, 3, 0), (1, 192, 108, 4, 0)]

            if USE_DMA_GATHER and not SKIP_GATHER:
                idbf = wp.tile([128, 128], bf)
                make_identity(nc, idbf[:])
            for t in range(NBT if not SKIP_GATHER else 0):
                sm = bp.tile([128, E], f32, tag="sum")
                if USE_DMA_GATHER:
                    # window-local int16 index streams from layout-prepped xg
                    xgt = cp.tile([128, 8 * S], i32, tag="xgt")
                    nc.sync.dma_start(out=xgt[:], in_=xg_d[t])
                    # pos = x + 1 + x//W  (x//W via range compares)
                    for thr in (W, 2 * W, 3 * W):
                        c = cp.tile([128, 8 * S], i32, tag="cmp")
                        nc.vector.tensor_scalar(out=c[:], in0=xgt[:],
                                                scalar1=float(thr), scalar2=None,
                                                op0=mybir.AluOpType.is_ge)
                        nc.vector.tensor_tensor(out=xgt[:], in0=xgt[:], in1=c[:],
                                                op=mybir.AluOpType.add)
                    nc.vector.tensor_scalar(out=xgt[:], in0=xgt[:], scalar1=1.0,
                                            scalar2=None, op0=mybir.AluOpType.add)
                    smp = smpool.tile([128, E], f32, tag="smp")
                    nmm = 0
                    for r in range(NWIN):
                        base = r * (W + 1)
                        hi = min(W + 1, NR - 1 - base)
                        ix = bp.tile([128, 8 * S], i16, tag="ix")
                        nc.vector.tensor_scalar(out=ix[:], in0=xgt[:],
                                                scalar1=-float(base), scalar2=0.0,
                                                op0=mybir.AluOpType.add,
                                                op1=mybir.AluOpType.max)
                        nc.vector.tensor_scalar(out=ix[:], in0=ix[:],
                                                scalar1=float(hi), scalar2=None,
                                                op0=mybir.AluOpType.min)
                        for j in range(S // GNS):
                            g = ap_.tile([128, GNS, EP], bf, tag="g")
                            nc.gpsimd.dma_gather(
                                out_ap=g[:],
                                in_ap=embp_d[base:base + hi + 1, :],
                                idxs_ap=ix[:, j * 128:(j + 1) * 128],
                                num_idxs=128 * GNS, num_idxs_reg=128 * GNS,
                                elem_size=EP, single_packet=False)
                            for u in range(GNS):
                                nc.tensor.matmul(
                                    out=smp[:], lhsT=idbf[:], rhs=g[:, u, 0:E],
                                    start=(nmm == 0),
                                    stop=(nmm == NWIN * S - 1))
                                nmm += 1
                    nc.vector.tensor_copy(out=sm[:], in_=smp[:])
                else:
                    accs = []
                    for r in range(ACC_ROT):
                        a = ap_.tile([128, E], f32, tag="acc")
                        for s in range(r, S, ACC_ROT):
                            nc.gpsimd.indirect_dma_start(
                                out=a[:], out_offset=None, in_=emb_d[:],
                                in_offset=bass.IndirectOffsetOnAxis(
                                    ap=xt[:, t, s:s + 1], axis=0),
                                compute_op=(mybir.AluOpType.bypass if s == r
                                            else mybir.AluOpType.add))
                        accs.append(a)
                    for step in (1, 2, 4):
                        for r in range(0, ACC_ROT, 2 * step):
                            if r + step < ACC_ROT:
                                nc.vector.tensor_tensor(
                                    out=accs[r][:], in0=accs[r][:],
                                    in1=accs[r + step][:],
                                    op=mybir.AluOpType.add)
                    nc.vector.tensor_copy(out=sm[:], in_=accs[0][:])
                ex0 = bp.tile([128, E], f32, tag="ex0")
                nc.gpsimd.indirect_dma_start(
                    out=ex0[:], out_offset=None, in_=emb_d[:],
                    in_offset=bass.IndirectOffsetOnAxis(
                        ap=xt[:, t, 0:1], axis=0))
                # masked sum = full sum + (cnt - S) * emb[PAD]; mean = /cnt
                nc.vector.scalar_tensor_tensor(
                    out=sm[:], in0=prs[:], scalar=ncd[:, t:t + 1], in1=sm[:],
                    op0=mybir.AluOpType.mult, op1=mybir.AluOpType.add)
                nc.vector.tensor_scalar(out=sm[:], in0=sm[:],
                                        scalar1=inv[:, t:t + 1], scalar2=None,
                                        op0=mybir.AluOpType.mult)
                # ---- feat transposes into featT -------------------------
                for (srcsel, c0, w, kt, r0) in tplan:
                    src = ex0 if srcsel == 0 else sm
                    tp = tpp.tile([128, 128], f32, tag="tp")
                    nc.tensor.transpose(out=tp[:w, :], in_=src[:, c0:c0 + w],
                                        identity=idf[:])
                    nc.vector.tensor_copy(
                        out=featT[r0:r0 + w, kt, t * 128:(t + 1) * 128],
                        in_=tp[:w, :])

            # ---- MLP ---------------------------------------------------
            for n in range(NCHUNKS if not SKIP_MLP else 0):
                ns = slice(n * NCH, (n + 1) * NCH)
                h1 = hp.tile([128, H1 // 128, NCH], bf, tag="h1")
                for m in range(H1 // 128):
                    ps = mp.tile([128, NCH], f32, tag="mm")
                    for kt, (k0, ksz) in enumerate(KT1):
                        kn = 128 if kt < 4 else ksz
                        nc.tensor.matmul(out=ps[:],
                                         lhsT=w1s[:kn, kt, m * 128:(m + 1) * 128],
                                         rhs=featT[:kn, kt, ns],
                                         start=(kt == 0), stop=(kt == len(KT1) - 1))
                    nc.vector.tensor_scalar(out=h1[:, m, :], in0=ps[:],
                                            scalar1=b1s[:, m:m + 1], scalar2=0.0,
                                            op0=mybir.AluOpType.add,
                                            op1=mybir.AluOpType.max)
                h2 = hp.tile([128, H2 // 128, NCH], bf, tag="h2")
                for m in range(H2 // 128):
                    ps = mp.tile([128, NCH], f32, tag="mm")
                    for kt in range(H1 // 128):
                        nc.tensor.matmul(out=ps[:],
                                         lhsT=w2s[:, kt, m * 128:(m + 1) * 128],
                                         rhs=h1[:, kt, :],
                                         start=(kt == 0), stop=(kt == H1 // 128 - 1))
                    nc.vector.tensor_scalar(out=h2[:, m, :], in0=ps[:],
                                            scalar1=b2s[:, m:m + 1], scalar2=0.0,
                                            op0=mybir.AluOpType.add,
                                            op1=mybir.AluOpType.max)
                # layer 3: activations stationary -> out [b, class]
                for bt in range(NCH // 128):
                    t = n * (NCH // 128) + bt
                    osb = op_.tile([128, C], f32, tag="osb")
                    for (c0, cw) in CCH:
                        ps3 = p3p.tile([128, 512], f32, tag="p3")
                        for kt in range(H2 // 128):
                            nc.tensor.matmul(
                                out=ps3[:, :cw],
                                lhsT=h2[:, kt, bt * 128:(bt + 1) * 128],
                                rhs=w3s[:, kt, c0:c0 + cw],
                                start=(kt == 0), stop=(kt == H2 // 128 - 1))
                        nc.vector.tensor_tensor(out=osb[:, c0:c0 + cw],
                                                in0=ps3[:, :cw],
                                                in1=bos[:, c0:c0 + cw],
                                                op=mybir.AluOpType.add)
                    nc.sync.dma_start(out=out_d[t * 128:(t + 1) * 128, :],
                                      in_=osb[:])
    nc.compile()
    _nc_cache["nc"] = nc
    return nc


def kernel(x, emb, W1, b1, W2, b2, W_out, b_out):
    x = np.asarray(x)
    emb = np.asarray(emb, dtype=np.float32)
    embb = emb.astype(bf16)
    # pack W1T rows to match the featT k-tile layout (zeros in the seam gap)
    w1t_nat = np.asarray(W1, np.float32).T  # [600, H1]
    w1t_pk = np.zeros((W1P, H1), np.float32)
    w1t_pk[0:300] = w1t_nat[0:300]          # ex0 rows (k-tiles 0-2)
    w1t_pk[320:384] = w1t_nat[300:364]      # mean[0:64] at tile2 partition 64
    w1t_pk[384:512] = w1t_nat[364:492]      # mean[64:192] -> tile 3
    w1t_pk[512:620] = w1t_nat[492:600]      # mean[192:300] -> tile 4
    w1t = np.ascontiguousarray(w1t_pk.astype(bf16))
    w2t = np.ascontiguousarray(np.asarray(W2, np.float32).T.astype(bf16))
    w3t = np.ascontiguousarray(np.asarray(W_out, np.float32).T.astype(bf16))
    b1s = np.ascontiguousarray(
        np.asarray(b1, np.float32).reshape(H1 // 128, 128).T)
    b2s = np.ascontiguousarray(
        np.asarray(b2, np.float32).reshape(H2 // 128, 128).T)
    bor = np.ascontiguousarray(
        np.broadcast_to(np.asarray(b_out, np.float32), (128, C)))
    padr = np.ascontiguousarray(
        np.broadcast_to(embb[PAD].astype(np.float32), (128, E)))
    xt = np.ascontiguousarray(x.T.astype(np.int32))  # [B, S]

    extra = {}
    if USE_DMA_GATHER:
        # padded table with zero rows interleaved every W vocab rows
        embp = np.zeros((NR, EP), np.float32)
        pos = np.arange(V) + 1 + np.arange(V) // W
        embp[pos, :E] = emb
        extra["embp"] = embp.astype(bf16)

    nc = build()
    in_maps = []
    for c in range(NCORES):
        m = {
            "xt": xt[c * BC:(c + 1) * BC],
            "emb": embb, "w1t": w1t, "w2t": w2t, "w3t": w3t,
            "b1s": b1s, "b2s": b2s, "bor": bor, "padr": padr,
            **extra,
        }
        if USE_DMA_GATHER:
            # xg[t, 16k+p16, s*8+q] = x[s, c*1024 + t*128 + q*16 + p16]
            xc = x[:, c * BC:(c + 1) * BC].astype(np.int32)  # [S, 1024]
            arr = xc.reshape(S, NBT, 8, 16).transpose(1, 3, 0, 2)  # [t,p16,s,q]
            arr = arr.reshape(NBT, 16, 8 * S)
            m["xg"] = np.ascontiguousarray(np.tile(arr, (1, 8, 1)))
        in_maps.append(m)
    res = run_bass_kernel_spmd(nc, in_maps, core_ids=list(range(NCORES)))
    return np.concatenate([res.results[c]["out"] for c in range(NCORES)],
                          axis=0)
